# revision 31
# baseline (speedup 1.0000x reference)
"""Trainium2 Bass kernel for nn_Attention_48095043781121 (v7).

Math (reference):
    q,k,v = x@Wq, x@Wk, x@Wv          (per head h: columns [64h, 64h+64))
    A     = softmax_j(q.k^T / 8)
    p     = relu(pos@Wp1+bp1)@Wp2+bp2
    P[b,h,i,j] = softmax_j(ph_i - ph_j + bh) = softmax_j(-ph_j) = w[b,h,j]
                 (i-part, bh AND the bp2 contribution all cancel in softmax)
    attn  = ((1-g)A + gP) / rowsum               rowsum == 1 exactly
    out   = attn @ v ;  y = concat_heads(out) @ Wo + bo

Per (b,h):  y += [(1-g_h)/r] * (E @ v_h) @ Wo_h  +  [g_h * (w @ v_h)] @ Wo_h
with E = exp(S/8), r[i] = sum_j E[i,j].  The second term is a constant row
(independent of the query i) -> computed once as `yb` and added on the host
along with bo during the unshard.

Sharding: 8 cores = 4 batches x 2 head-groups (heads 0-3 / 4-7); host sums
the two partial y (+ yb rows + bo) per batch.

v7 structure (all fp16 on the PE):
  - DMA waves: the ~330 GB/s per-core DMA bandwidth is shared across
    queues, so the q/k-projection-critical bytes (xT, Wq-m0, Wk-m0 ...)
    are sequenced first on every queue; Wv/Wo/pos trail behind.
  - E stored [j-part, i-free]; E@v uses v (augmented with a 1/(1-g) column)
    as stationary and E as moving, so output lands [feature-part, i-free]
    for the out-projection -- no PE transposes.
  - Scores for a head pair run as two concurrent row-group matmuls into the
    two banks of a [128, 2, 512] PSUM pair tile; ONE exp ACTIVATE per pair.
  - Normalization chain per head: cast the r/(1-g) PSUM row to an f16 row
    at partition 0, K=1-broadcast the RAW row over 64 partitions,
    reciprocal_approx_fast on the broadcast tile (base-0, one op), then one
    partition-shifted DVE multiply evacuates + scales + stacks the head.
  - Out-projection m0 half runs as soon as heads 0/1 are combined; y is
    written as two [256, 512] f16 halves; gwv/ybias trail at the end.
"""

import numpy as np
from contextlib import ExitStack

B, S, DIM, H, DH = 4, 512, 512, 8, 64
POS_DIM, PD8 = 3, 64
NCORES = 8
HGH = 4          # heads per head-group (per core)
HGF = HGH * DH   # feature columns per head-group = 256
KT = DIM // 128  # contraction tiles over model dim = 4
MT = HGF // 128  # feature tiles per head-group = 2
ST = S // 128    # token tiles = 4
DHA = DH + 1     # v columns padded: [v(64) | 1/(1-g)]
NWARM = 10       # HAM warmup matmuls bridging the input-DMA head

_CACHE = {}


def _build_program():
    import concourse.mybir as mybir
    import concourse.tile as tile
    from concourse import bacc
    from concourse.masks import make_identity

    F32 = mybir.dt.float32
    F32R = mybir.dt.float32r
    F16 = mybir.dt.float16
    AF = mybir.ActivationFunctionType
    ALU = mybir.AluOpType

    nc = bacc.Bacc(trn_type="TRN2", target_bir_lowering=False, debug=False)

    KB = KT * 128  # columns per m-block in the m-major weight layout
    xT_d = nc.dram_tensor("xT", [128, KT * S], F16, kind="ExternalInput")
    # Wq/Wk m-major: [128, MT, KT, 128] flattened
    wq_d = nc.dram_tensor("Wq", [128, MT * KB], F16, kind="ExternalInput")
    wk_d = nc.dram_tensor("Wk", [128, MT * KB], F16, kind="ExternalInput")
    wv_d = nc.dram_tensor("Wv", [128, KT * HGF], F16, kind="ExternalInput")
    wo_d = nc.dram_tensor("Wo", [128, MT * DIM], F16, kind="ExternalInput")
    # posP: [posT(512) | Wp1 padded to 4 | Wp2(64)]
    posP_d = nc.dram_tensor("posP", [POS_DIM, S + 4 + PD8], F32R, kind="ExternalInput")
    whP_d = nc.dram_tensor("whP", [PD8, HGH], F32R, kind="ExternalInput")
    # sclP: [bp1 | g]
    sclP_d = nc.dram_tensor("sclP", [HGH, 2], F32, kind="ExternalInput")
    vpad_d = nc.dram_tensor("vpad", [128, ST * HGH], F16, kind="ExternalInput")
    y_d = nc.dram_tensor("y", [S, DIM], F16, kind="ExternalOutput")
    yb_d = nc.dram_tensor("yb", [1, DIM], F32, kind="ExternalOutput")

    with tile.TileContext(nc) as tc, ExitStack() as ctx:
        sing = ctx.enter_context(tc.tile_pool(name="sing", bufs=1))
        scpool = ctx.enter_context(tc.tile_pool(name="scpool", bufs=2))
        ypool = ctx.enter_context(tc.tile_pool(name="ypool", bufs=2))
        # PSUM: 8 banks = ps_big 2x2 (score pairs + yps pairs) + ps_o 3x1 + ps_sel 1x1
        ps_big = ctx.enter_context(tc.tile_pool(name="ps_big", bufs=2, space="PSUM"))
        ps_o = ctx.enter_context(tc.tile_pool(name="ps_o", bufs=3, space="PSUM"))
        ps_sel = ctx.enter_context(tc.tile_pool(name="ps_sel", bufs=1, space="PSUM"))

        # ---------------- input DMAs: kq-critical waves first ----------------
        xT = sing.tile([128, KT, S], F16)
        wq = sing.tile([128, MT, KT, 128], F16)
        wk = sing.tile([128, MT, KT, 128], F16)
        wv = sing.tile([128, KT, HGF], F16)
        wo = sing.tile([128, MT, DIM], F16)
        v_aug = sing.tile([128, ST, HGH, DHA], F16)
        posP = sing.tile([POS_DIM, S + 4 + PD8], F32R)
        whP = sing.tile([PD8, HGH], F32R)
        sclP = sing.tile([HGH, 2], F32)

        xT_r, wq_r, wk_r = xT_d.ap(), wq_d.ap(), wk_d.ap()
        # sync: xT-k01, xT-k23, [y outputs later]
        nc.sync.dma_start(out=xT[:, 0:2, :], in_=xT_r[:, 0 : 2 * S])
        nc.sync.dma_start(out=xT[:, 2:KT, :], in_=xT_r[:, 2 * S :])
        # scalar: Wq-m0, Wk-m0, Wq-m1, Wk-m1
        nc.scalar.dma_start(out=wq[:, 0], in_=wq_r[:, 0:KB])
        nc.scalar.dma_start(out=wk[:, 0], in_=wk_r[:, 0:KB])
        nc.scalar.dma_start(out=wq[:, 1], in_=wq_r[:, KB:])
        nc.scalar.dma_start(out=wk[:, 1], in_=wk_r[:, KB:])
        # gpsimd: wv, pos-small, wo
        nc.gpsimd.dma_start(out=wv, in_=wv_d.ap())
        nc.gpsimd.dma_start(out=posP, in_=posP_d.ap())
        nc.gpsimd.dma_start(out=whP, in_=whP_d.ap())
        nc.gpsimd.dma_start(out=sclP, in_=sclP_d.ap())
        nc.gpsimd.dma_start(
            out=v_aug[:, :, :, DH : DH + 1],
            in_=vpad_d.ap().rearrange("p (t h) -> p t h", h=HGH)[:, :, :, None],
        )
        nc.gpsimd.dma_start(out=wo, in_=wo_d.ap())

        # ---------------- constants + HAM warmup ----------------
        warm = sing.tile([128, 512], F16)
        nc.vector.memset(warm, 0.25)
        ones1_f = sing.tile([1, DH], F32)
        nc.vector.memset(ones1_f, 1.0)
        ones64h = sing.tile([1, DH], F16)
        nc.vector.tensor_copy(ones64h, ones1_f)
        ident = sing.tile([128, 128], F32)
        make_identity(nc, ident)
        with nc.named_scope("warmup"):
            for _ in range(NWARM):
                wps = ps_o.tile([128, 512], F32, tag="o")
                nc.tensor.matmul(wps, warm[:, 0:128], warm, start=True, stop=True)

        # ---------------- tiles ----------------
        # kqT[:, m, 0, :] = k features (m-block), kqT[:, m, 1, :] = q
        kqT = sing.tile([128, MT, 2, S], F16)
        e_sb = sing.tile([128, ST, HGH, S], F16)
        oT = sing.tile([128, MT, S], F16)
        gwv_cols = sing.tile([128, MT], F16)
        r16s = [sing.tile([1, S], F16, name=f"r16_{h}") for h in range(HGH)]
        ups = [None] * HGH

        def proj_kq(m, evac_engines):
            kps = ps_o.tile([128, S], F32, tag="o", name=f"kp{m}")
            qps = ps_o.tile([128, S], F32, tag="o", name=f"qp{m}")
            for kks in (range(0, 2), range(2, KT)):
                for w, ps in ((wk, kps), (wq, qps)):
                    for kk in kks:
                        nc.tensor.matmul(
                            ps,
                            w[:, m, kk, :],
                            xT[:, kk, :],
                            start=(kk == 0),
                            stop=(kk == KT - 1),
                        )
            for idx, ps in ((0, kps), (1, qps)):
                if evac_engines[idx] == "act":
                    nc.scalar.activation(kqT[:, m, idx, :], ps, AF.Copy)
                else:
                    nc.vector.tensor_copy(kqT[:, m, idx, :], ps)

        def proj_v():
            for tp in range(ST // 2):
                pair = ps_o.tile([128, 2, HGF], F32, tag="o", name=f"vp{tp}")
                for half in range(2):
                    tt = 2 * tp + half
                    for kk in range(KT):
                        nc.tensor.matmul(
                            pair[:, half, :],
                            xT[:, kk, 128 * tt : 128 * (tt + 1)],
                            wv[:, kk, :],
                            start=(kk == 0),
                            stop=(kk == KT - 1),
                        )
                nc.vector.tensor_copy(
                    v_aug[:, 2 * tp : 2 * tp + 2, :, 0:DH],
                    pair.rearrange("p a (h c) -> p a h c", c=DH),
                )

        def scores(m):
            for jt in range(ST):
                pair = ps_big.tile([128, 2, S], F32, tag="big", name=f"sc{m}{jt}")
                for sub in range(2):
                    off = 64 * sub
                    nc.tensor.matmul(
                        pair[:, sub, :],
                        kqT[off : off + 64, m, 0, 128 * jt : 128 * (jt + 1)],
                        kqT[off : off + 64, m, 1, :],
                        start=True,
                        stop=True,
                    )
                nc.scalar.activation(
                    e_sb[:, jt, 2 * m : 2 * m + 2, :], pair, AF.Exp, scale=0.125
                )

        def mm2(h, row_engine):
            u = ps_o.tile([DHA, S], F32, tag="o", name=f"ups{h}")
            ups[h] = u
            for jt in range(ST):
                nc.tensor.matmul(
                    u,
                    v_aug[:, jt, h, :],
                    e_sb[:, jt, h, :],
                    start=(jt == 0),
                    stop=(jt == ST - 1),
                )
            # raw r/(1-g) row -> f16 row at partition 0
            if row_engine == "act":
                nc.scalar.activation(r16s[h], u[DH : DH + 1, :], AF.Copy)
            else:
                nc.vector.tensor_copy(r16s[h], u[DH : DH + 1, :])

        def sel_combine(h, via_gps=False):
            # broadcast RAW row over 64 partitions, reciprocal on the
            # broadcast tile (base-0 -> approx_fast is safe), then scale
            sc_ps = ps_sel.tile([DH, S], F32, tag="sel", name=f"scp{h}")
            nc.tensor.matmul(sc_ps, ones64h, r16s[h], start=True, stop=True)
            scINV = scpool.tile([DH, S], F32, tag="sc")
            nc.vector.reciprocal_approx_fast(scINV, sc_ps)
            off = 64 * (h % 2)
            if via_gps:
                # evacuate ups on ACT, multiply on GpSimd (frees DVE)
                upsSB = sing.tile([DH, S], F16, name=f"upsb{h}")
                nc.scalar.activation(upsSB, ups[h][0:DH, :], AF.Copy)
                nc.gpsimd.tensor_tensor(
                    out=oT[off : off + 64, h // 2, :],
                    in0=upsSB,
                    in1=scINV,
                    op=ALU.mult,
                )
            else:
                nc.vector.tensor_tensor(
                    out=oT[off : off + 64, h // 2, :],
                    in0=ups[h][0:DH, :],
                    in1=scINV,
                    op=ALU.mult,
                )

        def gwv_all():
            gwp = ps_o.tile([DH, HGH], F32, tag="o", name="gwp")
            for h in range(HGH):
                for jt in range(ST):
                    nc.tensor.matmul(
                        gwp[:, h : h + 1],
                        v_aug[:, jt, h, 0:DH],
                        wj[:, jt, h : h + 1],
                        start=(jt == 0),
                        stop=(jt == ST - 1),
                    )
            # even heads -> rows 0-63, odd heads -> rows 64-127
            g4 = gwp.rearrange("p (m two) -> p m two", two=2)
            nc.vector.tensor_copy(gwv_cols[0:DH, :], g4[:, :, 0])
            nc.vector.tensor_copy(gwv_cols[DH:128, :], g4[:, :, 1])

        with nc.named_scope("proj_kq0"):
            proj_kq(0, ("act", "vec"))
        with nc.named_scope("scores0"):
            scores(0)

        # ---- position path (PE work slots into the exp-bound window)
        with nc.named_scope("pos_path"):
            p1ps = ps_sel.tile([4, S], F32, tag="sel")
            nc.tensor.matmul(
                p1ps, posP[:, S : S + 4], posP[:, 0:S], start=True, stop=True
            )
            p1 = sing.tile([4, S], F32R)
            nc.scalar.activation(p1, p1ps, AF.Relu, bias=sclP[:, 0:1])
            p2ps = ps_sel.tile([PD8, S], F32, tag="sel")
            nc.tensor.matmul(
                p2ps, posP[:, S + 4 :], p1[0:POS_DIM, :], start=True, stop=True
            )
            p2 = sing.tile([PD8, S], F32R)
            nc.vector.tensor_copy(p2, p2ps)  # bp2 cancels in the softmax
            phps = ps_sel.tile([HGH, S], F32, tag="sel")
            nc.tensor.matmul(phps, whP, p2, start=True, stop=True)
            phsb = sing.tile([HGH, S], F32)
            nc.vector.tensor_copy(phsb, phps)
        wj = sing.tile([128, ST, HGH], F16)

        with nc.named_scope("proj_kq1"):
            proj_kq(1, ("vec", "vec"))
        with nc.named_scope("proj_v"):
            proj_v()
        with nc.named_scope("scores1"):
            scores(1)

        with nc.named_scope("wj_tr"):
            # pos softmax exp deferred here so it never interrupts the
            # score-exp stream on ACT
            wexp = sing.tile([HGH, S], F32)
            wsum = sing.tile([HGH, 1], F32)
            nc.scalar.activation(wexp, phsb, AF.Exp, scale=-1.0, accum_out=wsum)
            winv = sing.tile([HGH, 1], F32)
            nc.vector.reciprocal(winv, wsum)
            gwin = sing.tile([HGH, 1], F32)
            nc.vector.tensor_mul(gwin, winv, sclP[:, 1:2])
            w_sb = sing.tile([HGH, S], F32)
            nc.vector.tensor_scalar_mul(w_sb, wexp, gwin)
            for jt in range(ST):
                wt = ps_sel.tile([128, HGH], F32, tag="sel", name=f"wt{jt}")
                nc.tensor.transpose(
                    wt, w_sb[:, 128 * jt : 128 * (jt + 1)], ident[0:HGH, 0:HGH]
                )
                nc.vector.tensor_copy(wj[:, jt, :], wt)

        with nc.named_scope("attn"):
            mm2(0, "vec")
            mm2(1, "vec")
            sel_combine(0)
            mm2(2, "act")
            sel_combine(1)
            mm2(3, "act")
            sel_combine(2)
            for _ in range(2):
                kw = ps_big.tile([128, 512], F32, tag="big", name="kw")
                nc.tensor.matmul(kw, warm[:, 0:128], warm, start=True, stop=True)
            sel_combine(3)
            for _ in range(2):
                kw = ps_big.tile([128, 512], F32, tag="big", name="kw")
                nc.tensor.matmul(kw, warm[:, 0:128], warm, start=True, stop=True)
            with nc.named_scope("gwv"):
                gwv_all()

        # ---------------- out-projection m0 half ----------------
        ypairs = [None, None]
        with nc.named_scope("outproj_m0"):
            for ip in range(2):
                ypairs[ip] = ps_big.tile(
                    [128, 2, DIM], F32, tag="big", name=f"yp{ip}"
                )
                for half in range(2):
                    it = 2 * ip + half
                    nc.tensor.matmul(
                        ypairs[ip][:, half, :],
                        oT[:, 0, 128 * it : 128 * (it + 1)],
                        wo[:, 0, :],
                        start=True,
                        stop=False,
                    )

        with nc.named_scope("ybias"):
            yb_ps = ps_o.tile([1, DIM], F32, tag="o", name="ybp")
            for m in range(MT):
                nc.tensor.matmul(
                    yb_ps,
                    gwv_cols[:, m : m + 1],
                    wo[:, m, :],
                    start=(m == 0),
                    stop=(m == MT - 1),
                )
            ybsb = sing.tile([1, DIM], F32)
            nc.vector.tensor_copy(ybsb, yb_ps)
            nc.sync.dma_start(out=yb_d.ap(), in_=ybsb)

        # ---------------- out-projection m1 half + y DMAs ----------------
        with nc.named_scope("outproj_m1"):
            for ip in range(2):
                for half in range(2):
                    it = 2 * ip + half
                    nc.tensor.matmul(
                        ypairs[ip][:, half, :],
                        oT[:, 1, 128 * it : 128 * (it + 1)],
                        wo[:, 1, :],
                        start=False,
                        stop=True,
                    )
                ysb = ypool.tile([128, 2, DIM], F16, tag="y")
                if ip == 0:
                    nc.scalar.activation(ysb, ypairs[ip], AF.Copy)
                else:
                    nc.vector.tensor_copy(ysb, ypairs[ip])
                nc.sync.dma_start(
                    out=y_d.ap()
                    .rearrange("(a p) d -> p a d", p=128)[:, 2 * ip : 2 * ip + 2, :],
                    in_=ysb,
                )

    nc.compile()
    return nc


def _get_program():
    if "nc" not in _CACHE:
        _CACHE["nc"] = _build_program()
    return _CACHE["nc"]


def _ktile(a, dtype=np.float16):
    # [K*128, n] -> [128, K*n] (per-partition-contiguous k-tile layout)
    k = a.shape[0] // 128
    return np.ascontiguousarray(
        a.reshape(k, 128, a.shape[1]).transpose(1, 0, 2).reshape(128, -1).astype(dtype)
    )


def _ktile_m(a):
    # k-tile layout reordered m-major: [128, MT, KT, 128]
    t = _ktile(a).reshape(128, KT, MT, 128)
    return np.ascontiguousarray(t.transpose(0, 2, 1, 3).reshape(128, -1))


def _make_in_maps(inputs):
    f = lambda a: np.ascontiguousarray(np.asarray(a), dtype=np.float32)
    x = f(inputs["x"])
    pos = f(inputs["pos"])
    Wq, Wk, Wv, Wo = f(inputs["Wq"]), f(inputs["Wk"]), f(inputs["Wv"]), f(inputs["Wo"])
    Wp1, bp1 = f(inputs["Wp1"]), f(inputs["bp1"])
    Wh, gate = f(inputs["Wh"]), f(inputs["gate"])
    gfull = 1.0 / (1.0 + np.exp(-gate.astype(np.float64)))  # sigmoid on host

    wp1_pad = np.zeros((POS_DIM, 4), np.float32)
    wp1_pad[:, :POS_DIM] = Wp1
    bp1_pad = np.zeros((HGH,), np.float32)
    bp1_pad[:POS_DIM] = bp1
    Wp2 = f(inputs["Wp2"])  # [3, 64]; bp2 cancels in the softmax

    in_maps = []
    for c in range(NCORES):
        b, hg = c // 2, c % 2
        cs = slice(HGF * hg, HGF * (hg + 1))
        g = gfull[HGH * hg : HGH * (hg + 1)].astype(np.float32)
        inv1mg = (1.0 / (1.0 - g.astype(np.float64))).astype(np.float32)
        posP = np.concatenate(
            [np.ascontiguousarray(pos[b].T), wp1_pad, Wp2], axis=1
        ).astype(np.float32)
        sclP = np.zeros((HGH, 2), np.float32)
        sclP[:, 0] = bp1_pad
        sclP[:, 1] = g
        vpad = np.tile(inv1mg.astype(np.float16)[None, :], (128, ST)).reshape(128, -1)
        in_maps.append(
            {
                "xT": _ktile(x[b].T),
                "Wq": _ktile_m(Wq[:, cs]),
                "Wk": _ktile_m(Wk[:, cs]),
                "Wv": _ktile(Wv[:, cs]),
                "Wo": _ktile(Wo[cs, :]),
                "posP": posP,
                "whP": np.ascontiguousarray(Wh[:, HGH * hg : HGH * (hg + 1)]),
                "sclP": sclP,
                "vpad": np.ascontiguousarray(vpad),
            }
        )
    return in_maps


def run(inputs, trace=False):
    """Run on 8 NeuronCores; returns (out [B,S,DIM] fp32, BassKernelResults)."""
    from concourse.bass_utils import run_bass_kernel_spmd

    nc = _get_program()
    in_maps = _make_in_maps(inputs)
    res = run_bass_kernel_spmd(
        nc, in_maps, core_ids=list(range(NCORES)), trace=trace
    )
    bo = np.asarray(inputs["bo"], np.float32)
    out = np.empty((B, S, DIM), np.float32)
    for b in range(B):
        r0, r1 = res.results[2 * b], res.results[2 * b + 1]
        out[b] = (
            r0["y"].astype(np.float32)
            + r1["y"].astype(np.float32)
            + r0["yb"]
            + r1["yb"]
            + bo[None, :]
        )
    return out, res


def kernel(**inputs):
    out, _ = run(inputs, trace=False)
    return out


# revision 32
# speedup vs baseline: 1.2556x; 1.2556x over previous
"""Trainium2 Bass kernel for nn_Attention_48095043781121 (v7).

Math (reference):
    q,k,v = x@Wq, x@Wk, x@Wv          (per head h: columns [64h, 64h+64))
    A     = softmax_j(q.k^T / 8)
    p     = relu(pos@Wp1+bp1)@Wp2+bp2
    P[b,h,i,j] = softmax_j(ph_i - ph_j + bh) = softmax_j(-ph_j) = w[b,h,j]
                 (i-part, bh AND the bp2 contribution all cancel in softmax)
    attn  = ((1-g)A + gP) / rowsum               rowsum == 1 exactly
    out   = attn @ v ;  y = concat_heads(out) @ Wo + bo

Per (b,h):  y += [(1-g_h)/r] * (E @ v_h) @ Wo_h  +  [g_h * (w @ v_h)] @ Wo_h
with E = exp(S/8), r[i] = sum_j E[i,j].  The second term is a constant row
(independent of the query i) -> computed once as `yb` and added on the host
along with bo during the unshard.

Sharding: 8 cores = 4 batches x 2 head-groups (heads 0-3 / 4-7); host sums
the two partial y (+ yb rows + bo) per batch.

v7 structure (all fp16 on the PE):
  - DMA waves: the ~330 GB/s per-core DMA bandwidth is shared across
    queues, so the q/k-projection-critical bytes (xT, Wq-m0, Wk-m0 ...)
    are sequenced first on every queue; Wv/Wo/pos trail behind.
  - E stored [j-part, i-free]; E@v uses v (augmented with a 1/(1-g) column)
    as stationary and E as moving, so output lands [feature-part, i-free]
    for the out-projection -- no PE transposes.
  - Scores for a head pair run as two concurrent row-group matmuls into the
    two banks of a [128, 2, 512] PSUM pair tile; ONE exp ACTIVATE per pair.
  - Normalization chain per head: cast the r/(1-g) PSUM row to an f16 row
    at partition 0, K=1-broadcast the RAW row over 64 partitions,
    reciprocal_approx_fast on the broadcast tile (base-0, one op), then one
    partition-shifted DVE multiply evacuates + scales + stacks the head.
  - Out-projection m0 half runs as soon as heads 0/1 are combined; y is
    written as two [256, 512] f16 halves; gwv/ybias trail at the end.
"""

import numpy as np
from contextlib import ExitStack

B, S, DIM, H, DH = 4, 512, 512, 8, 64
POS_DIM, PD8 = 3, 64
NCORES = 8
HGH = 4          # heads per head-group (per core)
HGF = HGH * DH   # feature columns per head-group = 256
KT = DIM // 128  # contraction tiles over model dim = 4
MT = HGF // 128  # feature tiles per head-group = 2
ST = S // 128    # token tiles = 4
DHA = DH + 1     # v columns padded: [v(64) | 1/(1-g)]
NWARM = 10       # HAM warmup matmuls bridging the input-DMA head

_CACHE = {}


def _build_program():
    import concourse.mybir as mybir
    import concourse.tile as tile
    from concourse import bacc
    from concourse.masks import make_identity

    F32 = mybir.dt.float32
    F32R = mybir.dt.float32r
    F16 = mybir.dt.float16
    AF = mybir.ActivationFunctionType
    ALU = mybir.AluOpType

    nc = bacc.Bacc(trn_type="TRN2", target_bir_lowering=False, debug=False)

    KB = KT * 128  # columns per m-block in the m-major weight layout
    xT_d = nc.dram_tensor("xT", [128, KT * S], F16, kind="ExternalInput")
    # Wq/Wk m-major: [128, MT, KT, 128] flattened
    wq_d = nc.dram_tensor("Wq", [128, MT * KB], F16, kind="ExternalInput")
    wk_d = nc.dram_tensor("Wk", [128, MT * KB], F16, kind="ExternalInput")
    wv_d = nc.dram_tensor("Wv", [128, KT * HGF], F16, kind="ExternalInput")
    wo_d = nc.dram_tensor("Wo", [128, MT * DIM], F16, kind="ExternalInput")
    # posP: [posT(512) | Wp1 padded to 4 | Wp2(64)]
    posP_d = nc.dram_tensor("posP", [POS_DIM, S + 4 + PD8], F32R, kind="ExternalInput")
    whP_d = nc.dram_tensor("whP", [PD8, HGH], F32R, kind="ExternalInput")
    # sclP: [bp1 | g]
    sclP_d = nc.dram_tensor("sclP", [HGH, 2], F32, kind="ExternalInput")
    vpad_d = nc.dram_tensor("vpad", [128, ST * HGH], F16, kind="ExternalInput")
    y_d = nc.dram_tensor("y", [S, DIM], F16, kind="ExternalOutput")
    yb_d = nc.dram_tensor("yb", [1, DIM], F32, kind="ExternalOutput")

    with tile.TileContext(nc) as tc, ExitStack() as ctx:
        sing = ctx.enter_context(tc.tile_pool(name="sing", bufs=1))
        scpool = ctx.enter_context(tc.tile_pool(name="scpool", bufs=2))
        ypool = ctx.enter_context(tc.tile_pool(name="ypool", bufs=2))
        # PSUM: 8 banks = ps_big 2x2 (score pairs + yps pairs) + ps_o 3x1 + ps_sel 1x1
        ps_big = ctx.enter_context(tc.tile_pool(name="ps_big", bufs=2, space="PSUM"))
        ps_o = ctx.enter_context(tc.tile_pool(name="ps_o", bufs=3, space="PSUM"))
        ps_sel = ctx.enter_context(tc.tile_pool(name="ps_sel", bufs=1, space="PSUM"))

        # ---------------- input DMAs: kq-critical waves first ----------------
        xT = sing.tile([128, KT, S], F16)
        wq = sing.tile([128, MT, KT, 128], F16)
        wk = sing.tile([128, MT, KT, 128], F16)
        wv = sing.tile([128, KT, HGF], F16)
        wo = sing.tile([128, MT, DIM], F16)
        v_aug = sing.tile([128, ST, HGH, DHA], F16)
        posP = sing.tile([POS_DIM, S + 4 + PD8], F32R)
        whP = sing.tile([PD8, HGH], F32R)
        sclP = sing.tile([HGH, 2], F32)

        xT_r, wq_r, wk_r = xT_d.ap(), wq_d.ap(), wk_d.ap()
        # sync: xT-k01, xT-k23, [y outputs later]
        nc.sync.dma_start(out=xT[:, 0:2, :], in_=xT_r[:, 0 : 2 * S])
        nc.sync.dma_start(out=xT[:, 2:KT, :], in_=xT_r[:, 2 * S :])
        # scalar: Wq-m0, Wk-m0, Wq-m1, Wk-m1
        nc.scalar.dma_start(out=wq[:, 0], in_=wq_r[:, 0:KB])
        nc.scalar.dma_start(out=wk[:, 0], in_=wk_r[:, 0:KB])
        nc.scalar.dma_start(out=wq[:, 1], in_=wq_r[:, KB:])
        nc.scalar.dma_start(out=wk[:, 1], in_=wk_r[:, KB:])
        # gpsimd: wv, pos-small, wo
        nc.gpsimd.dma_start(out=wv, in_=wv_d.ap())
        nc.gpsimd.dma_start(out=posP, in_=posP_d.ap())
        nc.gpsimd.dma_start(out=whP, in_=whP_d.ap())
        nc.gpsimd.dma_start(out=sclP, in_=sclP_d.ap())
        nc.gpsimd.dma_start(
            out=v_aug[:, :, :, DH : DH + 1],
            in_=vpad_d.ap().rearrange("p (t h) -> p t h", h=HGH)[:, :, :, None],
        )
        nc.gpsimd.dma_start(out=wo, in_=wo_d.ap())

        # ---------------- constants + HAM warmup ----------------
        warm = sing.tile([128, 512], F16)
        nc.vector.memset(warm, 0.25)
        ones1_f = sing.tile([1, DH], F32)
        nc.vector.memset(ones1_f, 1.0)
        ones64h = sing.tile([1, DH], F16)
        nc.vector.tensor_copy(ones64h, ones1_f)
        ident = sing.tile([128, 128], F32)
        make_identity(nc, ident)
        with nc.named_scope("warmup"):
            for _ in range(NWARM):
                wps = ps_o.tile([128, 512], F32, tag="o")
                nc.tensor.matmul(wps, warm[:, 0:128], warm, start=True, stop=True)

        # ---------------- tiles ----------------
        # kqT[:, m, 0, :] = k features (m-block), kqT[:, m, 1, :] = q
        kqT = sing.tile([128, MT, 2, S], F16)
        e_sb = sing.tile([128, ST, HGH, S], F16)
        oT = sing.tile([128, MT, S], F16)
        gwv_cols = sing.tile([128, MT], F16)
        r16s = [sing.tile([1, S], F16, name=f"r16_{h}") for h in range(HGH)]
        ups = [None] * HGH

        def proj_kq(m, evac_engines):
            kps = ps_o.tile([128, S], F32, tag="o", name=f"kp{m}")
            qps = ps_o.tile([128, S], F32, tag="o", name=f"qp{m}")
            for kks in (range(0, 2), range(2, KT)):
                for w, ps in ((wk, kps), (wq, qps)):
                    for kk in kks:
                        nc.tensor.matmul(
                            ps,
                            w[:, m, kk, :],
                            xT[:, kk, :],
                            start=(kk == 0),
                            stop=(kk == KT - 1),
                        )
            for idx, ps in ((0, kps), (1, qps)):
                if evac_engines[idx] == "act":
                    nc.scalar.activation(kqT[:, m, idx, :], ps, AF.Copy)
                else:
                    nc.vector.tensor_copy(kqT[:, m, idx, :], ps)

        def proj_v():
            for tp in range(ST // 2):
                pair = ps_o.tile([128, 2, HGF], F32, tag="o", name=f"vp{tp}")
                for half in range(2):
                    tt = 2 * tp + half
                    for kk in range(KT):
                        nc.tensor.matmul(
                            pair[:, half, :],
                            xT[:, kk, 128 * tt : 128 * (tt + 1)],
                            wv[:, kk, :],
                            start=(kk == 0),
                            stop=(kk == KT - 1),
                        )
                nc.vector.tensor_copy(
                    v_aug[:, 2 * tp : 2 * tp + 2, :, 0:DH],
                    pair.rearrange("p a (h c) -> p a h c", c=DH),
                )

        def scores(m):
            for jt in range(ST):
                pair = ps_big.tile([128, 2, S], F32, tag="big", name=f"sc{m}{jt}")
                for sub in range(2):
                    off = 64 * sub
                    nc.tensor.matmul(
                        pair[:, sub, :],
                        kqT[off : off + 64, m, 0, 128 * jt : 128 * (jt + 1)],
                        kqT[off : off + 64, m, 1, :],
                        start=True,
                        stop=True,
                    )
                nc.scalar.activation(
                    e_sb[:, jt, 2 * m : 2 * m + 2, :], pair, AF.Exp, scale=0.125
                )

        def mm2(h, row_engine):
            u = ps_o.tile([DHA, S], F32, tag="o", name=f"ups{h}")
            ups[h] = u
            for jt in range(ST):
                nc.tensor.matmul(
                    u,
                    v_aug[:, jt, h, :],
                    e_sb[:, jt, h, :],
                    start=(jt == 0),
                    stop=(jt == ST - 1),
                )
            # raw r/(1-g) row -> f16 row at partition 0
            if row_engine == "act":
                nc.scalar.activation(r16s[h], u[DH : DH + 1, :], AF.Copy)
            else:
                nc.vector.tensor_copy(r16s[h], u[DH : DH + 1, :])

        def sel_combine(h, via_gps=False):
            # broadcast RAW row over 64 partitions, reciprocal on the
            # broadcast tile (base-0 -> approx_fast is safe), then scale
            sc_ps = ps_sel.tile([DH, S], F32, tag="sel", name=f"scp{h}")
            nc.tensor.matmul(sc_ps, ones64h, r16s[h], start=True, stop=True)
            scINV = scpool.tile([DH, S], F32, tag="sc")
            nc.vector.reciprocal_approx_fast(scINV, sc_ps)
            off = 64 * (h % 2)
            if via_gps:
                # evacuate ups on ACT, multiply on GpSimd (frees DVE)
                upsSB = sing.tile([DH, S], F16, name=f"upsb{h}")
                nc.scalar.activation(upsSB, ups[h][0:DH, :], AF.Copy)
                nc.gpsimd.tensor_tensor(
                    out=oT[off : off + 64, h // 2, :],
                    in0=upsSB,
                    in1=scINV,
                    op=ALU.mult,
                )
            else:
                nc.vector.tensor_tensor(
                    out=oT[off : off + 64, h // 2, :],
                    in0=ups[h][0:DH, :],
                    in1=scINV,
                    op=ALU.mult,
                )

        def gwv_all():
            gwp = ps_o.tile([DH, HGH], F32, tag="o", name="gwp")
            for h in range(HGH):
                for jt in range(ST):
                    nc.tensor.matmul(
                        gwp[:, h : h + 1],
                        v_aug[:, jt, h, 0:DH],
                        wj[:, jt, h : h + 1],
                        start=(jt == 0),
                        stop=(jt == ST - 1),
                    )
            # even heads -> rows 0-63, odd heads -> rows 64-127
            g4 = gwp.rearrange("p (m two) -> p m two", two=2)
            nc.vector.tensor_copy(gwv_cols[0:DH, :], g4[:, :, 0])
            nc.vector.tensor_copy(gwv_cols[DH:128, :], g4[:, :, 1])

        with nc.named_scope("proj_kq0"):
            proj_kq(0, ("act", "vec"))
        with nc.named_scope("scores0"):
            scores(0)

        # ---- position path (PE work slots into the exp-bound window)
        with nc.named_scope("pos_path"):
            p1ps = ps_sel.tile([4, S], F32, tag="sel")
            nc.tensor.matmul(
                p1ps, posP[:, S : S + 4], posP[:, 0:S], start=True, stop=True
            )
            p1 = sing.tile([4, S], F32R)
            nc.scalar.activation(p1, p1ps, AF.Relu, bias=sclP[:, 0:1])
            p2ps = ps_sel.tile([PD8, S], F32, tag="sel")
            nc.tensor.matmul(
                p2ps, posP[:, S + 4 :], p1[0:POS_DIM, :], start=True, stop=True
            )
            p2 = sing.tile([PD8, S], F32R)
            nc.vector.tensor_copy(p2, p2ps)  # bp2 cancels in the softmax
            phps = ps_sel.tile([HGH, S], F32, tag="sel")
            nc.tensor.matmul(phps, whP, p2, start=True, stop=True)
            phsb = sing.tile([HGH, S], F32)
            nc.vector.tensor_copy(phsb, phps)
        wj = sing.tile([128, ST, HGH], F16)

        with nc.named_scope("proj_kq1"):
            proj_kq(1, ("vec", "vec"))
        with nc.named_scope("proj_v"):
            proj_v()
        with nc.named_scope("scores1"):
            scores(1)


        with nc.named_scope("attn"):
            mm2(0, "vec")
            mm2(1, "vec")
            sel_combine(0)
            mm2(2, "act")
            sel_combine(1)
            mm2(3, "act")
            sel_combine(2)
            for _ in range(2):
                kw = ps_big.tile([128, 512], F32, tag="big", name="kw")
                nc.tensor.matmul(kw, warm[:, 0:128], warm, start=True, stop=True)
            sel_combine(3)
            for _ in range(2):
                kw = ps_big.tile([128, 512], F32, tag="big", name="kw")
                nc.tensor.matmul(kw, warm[:, 0:128], warm, start=True, stop=True)
            with nc.named_scope("gwv"):
                wexp = sing.tile([HGH, S], F32)
                wsum = sing.tile([HGH, 1], F32)
                nc.scalar.activation(wexp, phsb, AF.Exp, scale=-1.0, accum_out=wsum)
                winv = sing.tile([HGH, 1], F32)
                nc.vector.reciprocal(winv, wsum)
                gwin = sing.tile([HGH, 1], F32)
                nc.vector.tensor_mul(gwin, winv, sclP[:, 1:2])
                w_sb = sing.tile([HGH, S], F32)
                nc.vector.tensor_scalar_mul(w_sb, wexp, gwin)
                for jt in range(ST):
                    wt = ps_sel.tile([128, HGH], F32, tag="sel", name=f"wt{jt}")
                    nc.tensor.transpose(
                        wt, w_sb[:, 128 * jt : 128 * (jt + 1)], ident[0:HGH, 0:HGH]
                    )
                    nc.vector.tensor_copy(wj[:, jt, :], wt)
                gwv_all()

        # ---------------- out-projection m0 half ----------------
        ypairs = [None, None]
        with nc.named_scope("outproj_m0"):
            for ip in range(2):
                ypairs[ip] = ps_big.tile(
                    [128, 2, DIM], F32, tag="big", name=f"yp{ip}"
                )
                for half in range(2):
                    it = 2 * ip + half
                    nc.tensor.matmul(
                        ypairs[ip][:, half, :],
                        oT[:, 0, 128 * it : 128 * (it + 1)],
                        wo[:, 0, :],
                        start=True,
                        stop=False,
                    )

        with nc.named_scope("ybias"):
            yb_ps = ps_o.tile([1, DIM], F32, tag="o", name="ybp")
            for m in range(MT):
                nc.tensor.matmul(
                    yb_ps,
                    gwv_cols[:, m : m + 1],
                    wo[:, m, :],
                    start=(m == 0),
                    stop=(m == MT - 1),
                )
            ybsb = sing.tile([1, DIM], F32)
            nc.vector.tensor_copy(ybsb, yb_ps)
            nc.sync.dma_start(out=yb_d.ap(), in_=ybsb)

        # ---------------- out-projection m1 half + y DMAs ----------------
        with nc.named_scope("outproj_m1"):
            for ip in range(2):
                for half in range(2):
                    it = 2 * ip + half
                    nc.tensor.matmul(
                        ypairs[ip][:, half, :],
                        oT[:, 1, 128 * it : 128 * (it + 1)],
                        wo[:, 1, :],
                        start=False,
                        stop=True,
                    )
                ysb = ypool.tile([128, 2, DIM], F16, tag="y")
                if ip == 0:
                    nc.scalar.activation(ysb, ypairs[ip], AF.Copy)
                else:
                    nc.vector.tensor_copy(ysb, ypairs[ip])
                nc.sync.dma_start(
                    out=y_d.ap()
                    .rearrange("(a p) d -> p a d", p=128)[:, 2 * ip : 2 * ip + 2, :],
                    in_=ysb,
                )

    nc.compile()
    return nc


def _get_program():
    if "nc" not in _CACHE:
        _CACHE["nc"] = _build_program()
    return _CACHE["nc"]


def _ktile(a, dtype=np.float16):
    # [K*128, n] -> [128, K*n] (per-partition-contiguous k-tile layout)
    k = a.shape[0] // 128
    return np.ascontiguousarray(
        a.reshape(k, 128, a.shape[1]).transpose(1, 0, 2).reshape(128, -1).astype(dtype)
    )


def _ktile_m(a):
    # k-tile layout reordered m-major: [128, MT, KT, 128]
    t = _ktile(a).reshape(128, KT, MT, 128)
    return np.ascontiguousarray(t.transpose(0, 2, 1, 3).reshape(128, -1))


def _make_in_maps(inputs):
    f = lambda a: np.ascontiguousarray(np.asarray(a), dtype=np.float32)
    x = f(inputs["x"])
    pos = f(inputs["pos"])
    Wq, Wk, Wv, Wo = f(inputs["Wq"]), f(inputs["Wk"]), f(inputs["Wv"]), f(inputs["Wo"])
    Wp1, bp1 = f(inputs["Wp1"]), f(inputs["bp1"])
    Wh, gate = f(inputs["Wh"]), f(inputs["gate"])
    gfull = 1.0 / (1.0 + np.exp(-gate.astype(np.float64)))  # sigmoid on host

    wp1_pad = np.zeros((POS_DIM, 4), np.float32)
    wp1_pad[:, :POS_DIM] = Wp1
    bp1_pad = np.zeros((HGH,), np.float32)
    bp1_pad[:POS_DIM] = bp1
    Wp2 = f(inputs["Wp2"])  # [3, 64]; bp2 cancels in the softmax

    in_maps = []
    for c in range(NCORES):
        b, hg = c // 2, c % 2
        cs = slice(HGF * hg, HGF * (hg + 1))
        g = gfull[HGH * hg : HGH * (hg + 1)].astype(np.float32)
        inv1mg = (1.0 / (1.0 - g.astype(np.float64))).astype(np.float32)
        posP = np.concatenate(
            [np.ascontiguousarray(pos[b].T), wp1_pad, Wp2], axis=1
        ).astype(np.float32)
        sclP = np.zeros((HGH, 2), np.float32)
        sclP[:, 0] = bp1_pad
        sclP[:, 1] = g
        vpad = np.tile(inv1mg.astype(np.float16)[None, :], (128, ST)).reshape(128, -1)
        in_maps.append(
            {
                "xT": _ktile(x[b].T),
                "Wq": _ktile_m(Wq[:, cs]),
                "Wk": _ktile_m(Wk[:, cs]),
                "Wv": _ktile(Wv[:, cs]),
                "Wo": _ktile(Wo[cs, :]),
                "posP": posP,
                "whP": np.ascontiguousarray(Wh[:, HGH * hg : HGH * (hg + 1)]),
                "sclP": sclP,
                "vpad": np.ascontiguousarray(vpad),
            }
        )
    return in_maps


def run(inputs, trace=False):
    """Run on 8 NeuronCores; returns (out [B,S,DIM] fp32, BassKernelResults)."""
    from concourse.bass_utils import run_bass_kernel_spmd

    nc = _get_program()
    in_maps = _make_in_maps(inputs)
    res = run_bass_kernel_spmd(
        nc, in_maps, core_ids=list(range(NCORES)), trace=trace
    )
    bo = np.asarray(inputs["bo"], np.float32)
    out = np.empty((B, S, DIM), np.float32)
    for b in range(B):
        r0, r1 = res.results[2 * b], res.results[2 * b + 1]
        out[b] = (
            r0["y"].astype(np.float32)
            + r1["y"].astype(np.float32)
            + r0["yb"]
            + r1["yb"]
            + bo[None, :]
        )
    return out, res


def kernel(**inputs):
    out, _ = run(inputs, trace=False)
    return out
